# revision 2
# baseline (speedup 1.0000x reference)
"""NeuralGraphPool kernel for Trainium2 (8 NeuronCores, data-parallel over batch).

Computation (per molecule b):
    out[a, f] = max(atoms[a, f], max_{d: edges[a,d]>=0} atoms[edges[a,d], f])
                * (any edge valid ? 1 : 0)

Strategy (stage 2):
  - Shard batch B=256 across 8 cores (32 molecules each), processed in pairs.
  - Host precomputes per core: an fp16 atoms table (4096, 512), gather
    indices in dma_gather layout (int16; slot 0 = self, invalid edges
    replaced by the atom's own index -> max-idempotent), degree mask (128, 32).
  - Device: per molecule-pair one dma_gather pulls 2*(self + 8 neighbour)
    rows as (128, 18, 512) fp16; a DVE max-tree reduces the 9 slots of each
    molecule; ScalarE applies the degree mask during the fp16->f32 output
    copy; HWDGE stores the result.
"""

import numpy as np

import concourse.bass as bass
import concourse.bacc as bacc
import concourse.mybir as mybir
from concourse.tile import TileContext
from concourse.bass_utils import run_bass_kernel_spmd

# Problem constants (hardcoded per harness contract).
B, A, D, F = 256, 128, 8, 512
N_CORES = 8
BPC = B // N_CORES           # molecules per core (32)
S = D + 1                    # gather slots per atom (self + 8 neighbours)
PAIR = 2                     # molecules per gather/tree batch
NPAIR = BPC // PAIR          # 16
NI = PAIR * S * A            # gather indices per pair (2304)
IDX_COLS = NI // 16          # idx free-dim per pair (144)

_cached = {}


def _build_kernel():
    if "nc" in _cached:
        return _cached["nc"]
    nc = bacc.Bacc("TRN2", num_devices=N_CORES)
    f16 = mybir.dt.float16
    f32 = mybir.dt.float32
    MAX = mybir.AluOpType.max
    atoms16 = nc.declare_dram_parameter("atoms16", [BPC * A, F], f16, isOutput=False)
    gidx = nc.declare_dram_parameter("gidx", [128, NPAIR * IDX_COLS], mybir.dt.int16, isOutput=False)
    maskt = nc.declare_dram_parameter("maskt", [128, BPC], f32, isOutput=False)
    out = nc.declare_dram_parameter("out", [BPC * A, F], f32, isOutput=True)

    with TileContext(nc) as tc:
        with (
            tc.tile_pool(name="const", bufs=1) as cpool,
            tc.tile_pool(name="g", bufs=3) as gpool,
            tc.tile_pool(name="tmp", bufs=2) as tpool,
            tc.tile_pool(name="outp", bufs=3) as opool,
        ):
            idx_all = cpool.tile([128, NPAIR * IDX_COLS], mybir.dt.int16)
            nc.sync.dma_start(out=idx_all[:], in_=gidx[:])
            mask_all = cpool.tile([128, BPC], f32)
            nc.sync.dma_start(out=mask_all[:], in_=maskt[:])

            for p in range(NPAIR):
                g = gpool.tile([A, PAIR * S, F], f16)
                nc.gpsimd.dma_gather(
                    out_ap=g[:],
                    in_ap=atoms16[:],
                    idxs_ap=idx_all[:, p * IDX_COLS:(p + 1) * IDX_COLS],
                    num_idxs=NI,
                    num_idxs_reg=NI,
                    elem_size=F,
                    single_packet=False,
                )
                gv = g[:].rearrange("p (j s) f -> p j s f", s=S)
                # max-tree over the 9 slots of both molecules; slot 0 is self
                t = tpool.tile([A, PAIR, 4, F], f16)
                nc.vector.tensor_tensor(
                    out=t[:], in0=gv[:, :, 1:9:2, :], in1=gv[:, :, 2:9:2, :], op=MAX)
                u = tpool.tile([A, PAIR, 2, F], f16)
                nc.vector.tensor_tensor(
                    out=u[:], in0=t[:, :, 0:2, :], in1=t[:, :, 2:4, :], op=MAX)
                v = tpool.tile([A, PAIR, F], f16)
                nc.vector.tensor_tensor(
                    out=v[:], in0=u[:, :, 0, :], in1=u[:, :, 1, :], op=MAX)
                w = tpool.tile([A, PAIR, F], f16)
                nc.vector.tensor_tensor(out=w[:], in0=v[:], in1=gv[:, :, 0, :], op=MAX)
                # degree mask * fp16->f32 cast on the scalar engine
                o = opool.tile([A, PAIR, F], f32)
                for j in range(PAIR):
                    m = p * PAIR + j
                    nc.scalar.activation(
                        out=o[:, j, :], in_=w[:, j, :],
                        func=mybir.ActivationFunctionType.Copy,
                        bias=0.0, scale=mask_all[:, m:m + 1])
                dst = out[p * PAIR * A:(p + 1) * PAIR * A, :].rearrange(
                    "(j p) f -> p j f", p=A)
                nc.sync.dma_start(out=dst, in_=o[:])
    nc.compile()
    _cached["nc"] = nc
    return nc


def _host_prep(atoms, bonds, edges):
    """Build per-core input maps. atoms (B,A,F) f32; edges (B,A,D) int32."""
    del bonds  # unused by the layer
    a_idx = np.arange(A, dtype=np.int64)[None, :, None]            # (1,A,1)
    e = edges.astype(np.int64)
    valid = e >= 0
    e_fixed = np.where(valid, e, a_idx)                            # (B,A,D)
    mask = valid.any(axis=2).astype(np.float32)                    # (B,A)
    atoms16_full = atoms.astype(np.float16)                        # (B,A,F)

    in_maps = []
    for c in range(N_CORES):
        mol = slice(c * BPC, (c + 1) * BPC)
        at16 = np.ascontiguousarray(atoms16_full[mol].reshape(BPC * A, F))
        # global row index of slot s for atom a of molecule m: m*A + (a | edge)
        base = (np.arange(BPC, dtype=np.int64) * A)[:, None, None]  # (BPC,1,1)
        slots = np.concatenate(
            [np.broadcast_to(a_idx, (BPC, A, 1)), e_fixed[mol]], axis=2)  # (BPC,A,S)
        flat = (slots + base).astype(np.int16)                     # (BPC,A,S)
        # dma_gather position i = slot_global*128 + p -> (atom p, slot c);
        # slot_global enumerates PAIR*S slots of a molecule pair.
        per_pair = flat.transpose(0, 2, 1).reshape(NPAIR, NI)      # i = (m%2)*S*A + s*A + a
        idx_lay = per_pair.reshape(NPAIR, IDX_COLS, 16).transpose(0, 2, 1)
        idx16 = np.tile(idx_lay, (1, 8, 1)).transpose(1, 0, 2).reshape(128, NPAIR * IDX_COLS)
        idx16 = np.ascontiguousarray(idx16)
        mk = np.ascontiguousarray(mask[mol].T)                     # (A=128, BPC)
        in_maps.append({"atoms16": at16, "gidx": idx16, "maskt": mk})
    return in_maps


def kernel(atoms, bonds, edges, _want_timing=False, **_ignored):
    nc = _build_kernel()
    in_maps = _host_prep(np.asarray(atoms, dtype=np.float32), bonds,
                         np.asarray(edges, dtype=np.int32))
    res = run_bass_kernel_spmd(nc, in_maps, list(range(N_CORES)),
                               trace=_want_timing)
    outs = [res.results[c]["out"].reshape(BPC, A, F) for c in range(N_CORES)]
    full = np.concatenate(outs, axis=0)
    if _want_timing:
        return full, res
    return full



# revision 29
# speedup vs baseline: 273.3383x; 273.3383x over previous
"""NeuralGraphPool kernel for Trainium2 (8 NeuronCores, data-parallel over batch).

Computation (per molecule b):
    out[a, f] = max(atoms[a, f], max_{d: edges[a,d]>=0} atoms[edges[a,d], f])
                * (any edge valid ? 1 : 0)

Strategy (fp8 DoubleRow one-hot gather on PE):
  - Shard batch B=256 across 8 cores (32 molecules each), processed in pairs.
  - Host splits atoms into hi/lo fp8 (x ~= hi + lo), builds an fp8 one-hot
    table from edges (invalid edge -> own index, max-idempotent; all-zero
    column for degree-0 atoms so their slots give 0 and the mask folds away).
  - Device, per molecule: 8 DoubleRow matmuls (P^T@hi + P^T@lo at 0.5
    cycles/row, stride-0 broadcast weights) gather neighbour slots into f32
    PSUM. Real-HW constraints: GPSIMD cannot touch PSUM and any instruction
    may read at most ONE PSUM operand, so slot rounds either feed a DVE
    max-chain (TT psum x sbuf) seeded by the ScalarE-masked self row, or
    are copied to SBUF fp16 by ScalarE and pair-merged on DVE. fp16 result
    DMAs out (host casts f32).
"""

import numpy as np

import concourse.bass as bass
import concourse.bacc as bacc
import concourse.mybir as mybir
from concourse.tile import TileContext
from concourse.bass_utils import run_bass_kernel_spmd

# Problem constants (hardcoded per harness contract).
B, A, D, F = 256, 128, 8, 512
N_CORES = 8
BPC = B // N_CORES           # molecules per core (32)
NPAIR = BPC // 2

# engine assignment knobs --------------------------------------------
USE_DR = True                                  # fp8 DoubleRow gathers
# per-molecule slot-round patterns, cycled by molecule index:
# 'c' = DVE max-chain absorbs the round's 2 PSUM banks,
# 'a' = ScalarE copies the 2 banks to fp16 leaves (merged later on DVE)
EXIT_PATTERNS = ("acca", "acaa")

_cached = {}


def _build_kernel():
    if "nc" in _cached:
        return _cached["nc"]
    nc = bacc.Bacc("TRN2", num_devices=N_CORES)
    f16 = mybir.dt.float16
    f32 = mybir.dt.float32
    f8 = mybir.dt.float8e4
    MAX = mybir.AluOpType.max
    DR = mybir.MatmulPerfMode.DoubleRow

    def eng(name):
        return nc.vector if name == "dve" else nc.gpsimd

    atomspk = nc.declare_dram_parameter(
        "atomspk", [A, BPC * 2 * F], f8, isOutput=False)
    onehot = nc.declare_dram_parameter(
        "onehot", [128, BPC * D * 128], f8, isOutput=False)
    maskt = nc.declare_dram_parameter("maskt", [128, BPC], f32, isOutput=False)
    out = nc.declare_dram_parameter("out", [A, BPC * F], f16, isOutput=True)

    with TileContext(nc) as tc:
        with (
            tc.tile_pool(name="const", bufs=1) as cpool,
            tc.tile_pool(name="pk", bufs=3) as apool,
            tc.tile_pool(name="oh", bufs=3) as ohpool,
            tc.tile_pool(name="ps", bufs=1, space="PSUM") as pspool,
            tc.tile_pool(name="leaf", bufs=3) as lpool,
            tc.tile_pool(name="mid", bufs=3) as mpool,
            tc.tile_pool(name="outp", bufs=3) as opool,
        ):
            mask_all = cpool.tile([128, BPC], f32)

            for p in range(NPAIR):
                mA = 2 * p
                # hi/lo packed atoms for both molecules: [a, mol, j, f]
                pk = apool.tile([128, 2, 2, F], f8)
                nc.sync.dma_start(
                    out=pk[:],
                    in_=atomspk[:, mA * 2 * F:(mA + 2) * 2 * F].rearrange(
                        "p (m j f) -> p m j f", m=2, j=2))
                oh = ohpool.tile([128, 2, D, 128], f8)
                nc.sync.dma_start(
                    out=oh[:],
                    in_=onehot[:, mA * D * 128:(mA + 2) * D * 128].rearrange(
                        "p (m d a) -> p m d a", m=2, d=D))
                if p == 0:
                    nc.sync.dma_start(out=mask_all[:], in_=maskt[:])

                def mm(dst, w2, mol):
                    """Gather one slot: dst = w^T @ hi + w^T @ lo."""
                    if USE_DR:
                        nc.tensor.matmul(
                            out=dst, lhsT=w2, rhs=pk[:, mol, :, :],
                            start=True, stop=True, perf_mode=DR)
                    else:
                        nc.tensor.matmul(
                            out=dst, lhsT=w2[:, 0, :], rhs=pk[:, mol, 0, :],
                            start=True, stop=False)
                        nc.tensor.matmul(
                            out=dst, lhsT=w2[:, 1, :], rhs=pk[:, mol, 1, :],
                            start=False, stop=True)

                drain = p >= NPAIR - 1
                h = opool.tile([128, 2, F], f16)
                for mol in range(2):
                    mi = mA + mol
                    # masked self (fp16) seeds the DVE chain; deg-0 atoms
                    # get 0 via host-zeroed one-hot cols + mask scale
                    s16 = mpool.tile([128, F], f16, name="s16")
                    nc.scalar.activation(
                        out=s16[:], in_=pk[:, mol, 0, :],
                        func=mybir.ActivationFunctionType.Copy,
                        bias=0.0, scale=mask_all[:, mi:mi + 1])
                    # pattern: 'c' round -> DVE chain absorbs 2 banks;
                    # 'a' round -> Act copies 2 banks to fp16 leaves
                    pat = EXIT_PATTERNS[mi % len(EXIT_PATTERNS)]
                    lv = lpool.tile([128, 6, F], f16, name=f"lv{mol}")
                    chain = s16
                    napair = 0
                    for r in range(4):
                        ps = pspool.tile([128, 2, F], f32,
                                         name=f"ps{(4 * mol + r) % 4}")
                        for k in range(2):
                            w2 = (oh[:, mol, 2 * r + k, :].unsqueeze(1)
                                  .broadcast_to([128, 2, 128]))
                            mm(ps[:, k, :], w2, mol)
                        if pat[r] == "c":
                            v = mpool.tile([128, F], f16, name=f"v{r}a")
                            nc.vector.tensor_tensor(
                                out=v[:], in0=ps[:, 0, :], in1=chain[:],
                                op=MAX)
                            v2 = mpool.tile([128, F], f16, name=f"v{r}b")
                            nc.vector.tensor_tensor(
                                out=v2[:], in0=ps[:, 1, :], in1=v[:],
                                op=MAX)
                            chain = v2
                        else:
                            nc.scalar.activation(
                                out=lv[:, 2 * napair:2 * napair + 2, :],
                                in_=ps[:],
                                func=mybir.ActivationFunctionType.Copy,
                                bias=0.0, scale=1.0)
                            napair += 1

                    # DVE merges of the Act leaves (fp16, 2x mode): one
                    # strided op merges all pairs, then reduce to h
                    m1 = mpool.tile([128, 2, F], f16, name="m1")
                    if napair == 2:
                        nc.vector.tensor_tensor(
                            out=m1[:], in0=lv[:, 0:4:2, :],
                            in1=lv[:, 1:4:2, :], op=MAX)
                        m2 = mpool.tile([128, F], f16, name="m2")
                        nc.vector.tensor_tensor(
                            out=m2[:], in0=m1[:, 0, :], in1=m1[:, 1, :],
                            op=MAX)
                        nc.vector.tensor_tensor(
                            out=h[:, mol, :], in0=m2[:], in1=chain[:],
                            op=MAX)
                    else:
                        # 6 leaves: (0,1),(2,3) via one strided op; (4,5);
                        # then reduce with the chain
                        nc.vector.tensor_tensor(
                            out=m1[:], in0=lv[:, 0:4:2, :],
                            in1=lv[:, 1:4:2, :], op=MAX)
                        mp = mpool.tile([128, F], f16, name="mp")
                        nc.vector.tensor_tensor(
                            out=mp[:], in0=lv[:, 4, :], in1=lv[:, 5, :],
                            op=MAX)
                        m2 = mpool.tile([128, F], f16, name="m2")
                        nc.vector.tensor_tensor(
                            out=m2[:], in0=m1[:, 0, :], in1=m1[:, 1, :],
                            op=MAX)
                        m3 = mpool.tile([128, F], f16, name="m3")
                        nc.vector.tensor_tensor(
                            out=m3[:], in0=m2[:], in1=mp[:], op=MAX)
                        nc.vector.tensor_tensor(
                            out=h[:, mol, :], in0=m3[:], in1=chain[:],
                            op=MAX)

                nc.sync.dma_start(
                    out=out[:, mA * F:(mA + 2) * F].rearrange(
                        "p (m f) -> p m f", m=2),
                    in_=h[:])
    nc.compile()
    _cached["nc"] = nc
    return nc


def _host_prep(atoms, bonds, edges):
    """Build per-core input maps. atoms (B,A,F) f32; edges (B,A,D) int32."""
    del bonds  # unused by the layer
    f8np = mybir.dt.np(mybir.dt.float8e4)
    a_idx = np.arange(A, dtype=np.int64)[None, :, None]            # (1,A,1)
    e = edges.astype(np.int64)
    valid = e >= 0
    e_fixed = np.where(valid, e, a_idx)                            # (B,A,D)
    mask = valid.any(axis=2).astype(np.float32)                    # (B,A)
    hi = atoms.astype(f8np)                                        # (B,A,F) fp8
    lo = (atoms - hi.astype(np.float32)).astype(f8np)
    iota = np.arange(128, dtype=np.int64)

    in_maps = []
    for c in range(N_CORES):
        mol = slice(c * BPC, (c + 1) * BPC)
        # atomspk (A, BPC*2*F): [a, m, j, f], j=0 hi, j=1 lo
        pk = np.stack([hi[mol], lo[mol]], axis=2)                  # (BPC,A,2,F)
        pk = np.ascontiguousarray(
            pk.transpose(1, 0, 2, 3).reshape(A, BPC * 2 * F))
        # one-hot (128, BPC*D*128): [i, m*D*128 + d*128 + a]
        ohb = ((e_fixed[mol][:, :, :, None] == iota)
               & (mask[mol][:, :, None, None] > 0))               # (BPC,A,D,128)
        oh = np.ascontiguousarray(
            ohb.transpose(3, 0, 2, 1).reshape(128, BPC * D * 128)).astype(f8np)
        mk = np.ascontiguousarray(mask[mol].T)                     # (A=128, BPC)
        in_maps.append({"atomspk": pk, "onehot": oh, "maskt": mk})
    return in_maps


def kernel(atoms, bonds, edges, _want_timing=False, **_ignored):
    nc = _build_kernel()
    in_maps = _host_prep(np.asarray(atoms, dtype=np.float32), bonds,
                         np.asarray(edges, dtype=np.int32))
    res = run_bass_kernel_spmd(nc, in_maps, list(range(N_CORES)),
                               trace=False)
    outs = [
        res.results[c]["out"].reshape(A, BPC, F).transpose(1, 0, 2)
        for c in range(N_CORES)
    ]
    full = np.concatenate(outs, axis=0).astype(np.float32)
    if _want_timing:
        return full, res
    return full
